# revision 1
# baseline (speedup 1.0000x reference)
"""Trainium2 Bass kernel for nn_MultiHeadAttention_66322884984909.

Math (faithful to reference):
  Q = X @ W_Q.T reshaped (B, H, L, hd) via DIRECT reshape -> head h owns rows
  128h:128(h+1) of the projected (L, D) matrix, reinterpreted as (L=2048, hd=64).
  Heads are therefore sequence-parallel: the whole computation decomposes over
  the 32 (batch, head) pairs with no cross-pair coupling. 8 cores x 4 pairs.

Per pair (X_s = X[b, 128h:128h+128, :], shape (128, 1024)):
  Qf = X_s @ W_Q.T        (128, 1024)  -> Qh = Qf.reshape(2048, 64)
  S  = Qh @ Kh.T          (2048, 2048) causal-masked softmax (no scaling)
  O  = softmax(S) @ Vh    (2048, 64)
  Y  = O.reshape(128, 1024) @ W_O.T + b_O   -> out rows 128h:128(h+1) of batch b

No max-subtraction in softmax: logits ~ N(0, 64), |S| < 80 with overwhelming
probability, exp stays finite in fp32. Row sums come free as a 65th ones-column
appended to V in the P@V matmul. All matmuls run in fp32r (full-rate tensor
engine mode, ~1e-4 relative error).
"""

import numpy as np

import concourse.bass as bass
from concourse import bacc
import concourse.mybir as mybir
import concourse.tile as tile
from concourse.bass_utils import run_bass_kernel_spmd
from concourse.masks import make_identity

F32 = mybir.dt.float32
F32R = mybir.dt.float32r
EXP = mybir.ActivationFunctionType.Exp

B, L, D = 2, 2048, 1024
H, HD = 16, 64
NCORES = 8
PPC = 4  # pairs per core


def build_nc(repeat=1):
    nc = bacc.Bacc(trn_type="TRN2", target_bir_lowering=False, debug=False)

    xt = nc.declare_dram_parameter("xt", [PPC, 1024, 128], F32R, isOutput=False)
    wq = nc.declare_dram_parameter("wq", [1024, 1024], F32R, isOutput=False)   # W_Q.T
    wk = nc.declare_dram_parameter("wk", [1024, 1024], F32R, isOutput=False)   # W_K.T
    wv = nc.declare_dram_parameter("wv", [1024, 1024], F32R, isOutput=False)   # W_V.T
    wo = nc.declare_dram_parameter("wo", [64, 16 * 1024], F32R, isOutput=False)
    bias = nc.declare_dram_parameter("bias", [128, 1024], F32, isOutput=False)
    ones = nc.declare_dram_parameter("ones", [128, 16], F32R, isOutput=False)
    out = nc.declare_dram_parameter("out", [PPC, 128, 1024], F32, isOutput=True)
    qsh = nc.dram_tensor("qsh", [PPC // 2, 128, 2048], F32R)
    ksh = nc.dram_tensor("ksh", [PPC // 2, 128, 2048], F32R)
    vsh = nc.dram_tensor("vsh", [PPC, 128, 1040], F32R)

    with tile.TileContext(nc) as tc:
      for _rep in range(repeat):
        with (
            tc.tile_pool(name="consts", bufs=1) as consts,
            tc.tile_pool(name="headt", bufs=1) as headt,
            tc.tile_pool(name="mmps", bufs=4, space="PSUM") as mmps,
            tc.tile_pool(name="stps", bufs=1, space="PSUM") as stps,
            tc.tile_pool(name="onp", bufs=2) as onp,
            tc.tile_pool(name="ptp", bufs=4) as ptp,
            tc.tile_pool(name="rp", bufs=4) as rp,
            tc.tile_pool(name="yp", bufs=2) as ypool,
        ):
            bias_sb = consts.tile([128, 1024], F32)
            nc.sync.dma_start(out=bias_sb, in_=bias[:])
            ident_f = consts.tile([128, 128], F32)
            make_identity(nc, ident_f)
            ident = consts.tile([128, 128], F32R)
            nc.vector.tensor_copy(ident, ident_f)

            NG = PPC // 2
            qht2 = [headt.tile([128, 2048], F32R, tag=f"qht{g}", name=f"qht{g}")
                    for g in range(NG)]
            kht2 = [headt.tile([128, 2048], F32R, tag=f"kht{g}", name=f"kht{g}")
                    for g in range(NG)]
            vh = [headt.tile([128, 16 * 65], F32R, tag=f"vh{p}", name=f"vh{p}")
                  for p in range(PPC)]

            def emit_phases(xt_sb, pwork):
                """Projections + shuffles + transposes for all pairs."""
                for (phase_i, wparam, sh, is_v) in (
                        (0, wq, qsh, False), (1, wk, ksh, False),
                        (2, wv, vsh, True)):
                    with tc.tile_pool(name=f"pw{phase_i}", bufs=1) as pw:
                        w_sb = pw.tile([128, 8, 1024], F32R, tag="w",
                                       name=f"w{phase_i}")
                        for kc in range(8):
                            nc.sync.dma_start(
                                out=w_sb[:, kc, :],
                                in_=wparam.rearrange(
                                    "(c p) j -> p c j", p=128)[:, kc, :])
                      # loop groups inside the weight phase
                        for g2 in range(PPC // 2):
                          if is_v:
                              for ii in range(2):
                                  p = 2 * g2 + ii
                                  nat = pwork.tile([128, 1024], F32R, tag="natv",
                                                   bufs=2, name=f"natv{p}")
                                  for jh in range(2):
                                      ps = mmps.tile([128, 512], F32, tag="mm",
                                                     name="projps")
                                      for kc in range(8):
                                          nc.tensor.matmul(
                                              ps,
                                              lhsT=xt_sb[2 * g2 + ii][:, kc, :],
                                              rhs=w_sb[:, kc,
                                                       jh * 512:(jh + 1) * 512],
                                              start=(kc == 0), stop=(kc == 7),
                                          )
                                      nc.vector.tensor_copy(
                                          nat[:, jh * 512:(jh + 1) * 512], ps)
                                  shr = sh[p].rearrange(
                                      "(il pp2) (t j) -> t il pp2 j",
                                      il=8, t=16)[:, :, :, 0:64]
                                  nc.gpsimd.dma_start(out=shr, in_=nat[:])
                                  nc.sync.dma_start(out=vh[p][:], in_=sh[p])
                                  nc.gpsimd.dma_start(
                                      out=vh[p].rearrange(
                                          "q (b c) -> q b c", c=65)[:, :, 64],
                                      in_=ones[:])  # ones column at 65b+64
                          else:
                              # pair-interleaved (pp, pair, j2) scratch: bounce
                              # write streams 512B-contiguous runs
                              nat2 = pwork.tile([128, 16, 2, 64], F32R, tag="nat",
                                                bufs=3, name=f"nat2_{g2}_{phase_i}")
                              for ii in range(2):
                                  p = 2 * g2 + ii
                                  for jh in range(2):
                                      ps = mmps.tile([128, 512], F32, tag="mm",
                                                     name="projps")
                                      for kc in range(8):
                                          nc.tensor.matmul(
                                              ps,
                                              lhsT=xt_sb[2 * g2 + ii][:, kc, :],
                                              rhs=w_sb[:, kc,
                                                       jh * 512:(jh + 1) * 512],
                                              start=(kc == 0), stop=(kc == 7),
                                          )
                                      nc.vector.tensor_copy(
                                          nat2[:, 8 * jh:8 * (jh + 1), ii, :], ps)
                              shr = sh[g2].rearrange(
                                  "(il pp2) (t w j) -> t il pp2 w j",
                                  il=8, t=16, w=2)
                              nc.gpsimd.dma_start(out=shr, in_=nat2[:])
                              hh2 = pwork.tile([128, 2048], F32R, tag="hh",
                                               bufs=2, name=f"hh{g2}_{phase_i}")
                              nc.sync.dma_start(out=hh2[:], in_=sh[g2])
                              dst = qht2[g2] if phase_i == 0 else kht2[g2]
                              for bt in range(4):
                                  tb = stps.tile([128, 512], F32R, tag="stA",
                                                 name="trps")
                                  for j in range(4):
                                      ti = 4 * bt + j
                                      nc.tensor.transpose(
                                          tb[:, j * 128:(j + 1) * 128],
                                          hh2[:, ti * 128:(ti + 1) * 128],
                                          ident,
                                      )
                                  nc.scalar.copy(
                                      dst[:, bt * 512:(bt + 1) * 512], tb)

            def emit_attention(g):
                onorm2 = onp.tile([128, 2048], F32R, tag="onorm",
                                  name=f"onorm{g}")
                for a in range(4):
                    pvs = [mmps.tile([65, 512], F32, tag="mm",
                                     name=f"pv_{i}") for i in range(2)]
                    for gg in range(2 * a + 2):
                        sts = [stps.tile([128, 1024], F32, tag=t_,
                                         name=f"st{t_}")
                               for t_ in ("stA", "stB")]
                        for q2 in range(2):
                            bb = 2 * gg + q2
                            for i in range(2):
                                nc.tensor.matmul(
                                    sts[i][:, q2 * 512:(q2 + 1) * 512],
                                    lhsT=kht2[g][64 * i:64 * i + 64,
                                                 bb * 128:(bb + 1) * 128],
                                    rhs=qht2[g][64 * i:64 * i + 64,
                                                a * 512:(a + 1) * 512],
                                    start=True, stop=True,
                                )
                        for i in range(2):
                            pt = ptp.tile([128, 1024], F32R, tag="pt",
                                          name=f"pt_{i}")
                            nc.scalar.activation(pt, sts[i], EXP)
                            if gg >= 2 * a:  # diagonal: causal mask
                                r0 = 2 * (gg - 2 * a)
                                nc.gpsimd.affine_select(
                                    out=pt.rearrange("q (w j) -> q w j", w=2),
                                    in_=pt.rearrange("q (w j) -> q w j", w=2),
                                    compare_op=mybir.AluOpType.is_ge,
                                    fill=0.0,
                                    base=-128 * r0,
                                    pattern=[[-128, 2], [1, 512]],
                                    channel_multiplier=-1,
                                )
                            for q2 in range(2):
                                bb = 2 * gg + q2
                                nc.tensor.matmul(
                                    pvs[i],
                                    lhsT=vh[2 * g + i][:, bb * 65:bb * 65 + 65],
                                    rhs=pt[:, q2 * 512:(q2 + 1) * 512],
                                    start=(bb == 0), stop=(bb == 4 * a + 3),
                                )
                    for i in range(2):
                        r1 = rp.tile([1, 512], F32, tag="r1", name="r1_t")
                        nc.vector.reciprocal(r1, pvs[i][64:65, :])
                        rb = rp.tile([64, 512], F32, tag="rb", name="rb_t")
                        nc.gpsimd.partition_broadcast(rb, r1)
                        nc.vector.tensor_mul(
                            onorm2[64 * i:64 * i + 64, a * 512:(a + 1) * 512],
                            pvs[i][0:64, :], rb)

                return onorm2

            def emit_y(g, onorm2, wo_sb):
                # row-packed output projection for both pairs of the group
                onorm_r = onorm2.rearrange("q (i t) -> q t i", t=16)
                ysbs = [ypool.tile([128, 1024], F32, tag="ysb",
                                   name=f"ysb{g}_{i}") for i in range(2)]
                for jh in range(2):
                    yps = [mmps.tile([128, 512], F32, tag="mm",
                                     name=f"ypsum_{i}") for i in range(2)]
                    for t in range(16):
                        for i in range(2):
                            nc.tensor.matmul(
                                yps[i],
                                lhsT=onorm_r[64 * i:64 * i + 64, t, :],
                                rhs=wo_sb[64 * i:64 * i + 64,
                                          t * 1024 + jh * 512:
                                          t * 1024 + (jh + 1) * 512],
                                start=(t == 0), stop=(t == 15),
                            )
                    for i in range(2):
                        nc.vector.tensor_add(
                            ysbs[i][:, jh * 512:(jh + 1) * 512], yps[i],
                            bias_sb[:, jh * 512:(jh + 1) * 512])
                for i in range(2):
                    nc.sync.dma_start(out=out[2 * g + i], in_=ysbs[i])

            # pipeline: group-0 phases; group-1 phases overlap group-0
            # attention (DMA is idle during attention)
            with tile.TileContext.tile_pool(tc, name="xtp", bufs=1) as xtp, \
                 tile.TileContext.tile_pool(tc, name="pwork", bufs=1) as pwork:
                xt_sb = []
                for p in range(PPC):
                    t = xtp.tile([128, 8, 128], F32R, tag=f"xt{p}", name=f"xtsb{p}")
                    nc.scalar.dma_start(
                        out=t, in_=xt[p].rearrange("(c p) i -> p c i", p=128))
                    xt_sb.append(t)
                emit_phases(xt_sb, pwork)

            with tc.tile_pool(name="p2", bufs=1) as p2:
                wo_sb = p2.tile([128, 16 * 1024], F32R, tag="wo")
                for wc in range(8):
                    nc.sync.dma_start(out=wo_sb[0:64, wc * 2048:(wc + 1) * 2048],
                                      in_=wo[:, wc * 2048:(wc + 1) * 2048])
                    nc.vector.tensor_copy(
                        wo_sb[64:128, wc * 2048:(wc + 1) * 2048],
                        wo_sb[0:64, wc * 2048:(wc + 1) * 2048])
                for g in range(NG):
                    onorm2 = emit_attention(g)
                    emit_y(g, onorm2, wo_sb)

    nc.finalize()




    return nc


def _host_prep(input_seq_embs, W_Q, W_K, W_V, W_O, b_O):
    X = np.asarray(input_seq_embs, dtype=np.float32)
    WQ = np.asarray(W_Q, dtype=np.float32)
    WK = np.asarray(W_K, dtype=np.float32)
    WV = np.asarray(W_V, dtype=np.float32)
    WO = np.asarray(W_O, dtype=np.float32)
    bO = np.asarray(b_O, dtype=np.float32)

    wq_arr = np.ascontiguousarray(WQ.T)
    wk_arr = np.ascontiguousarray(WK.T)
    wv_arr = np.ascontiguousarray(WV.T)
    # wo[j2, 1024 t + jo] = W_O.T[64 t + j2, jo]
    wo_arr = np.ascontiguousarray(
        WO.T.reshape(16, 64, 1024).transpose(1, 0, 2).reshape(64, 16 * 1024))
    bias_arr = np.ascontiguousarray(
        np.broadcast_to(bO, (128, 1024)).astype(np.float32))

    in_maps = []
    for c in range(NCORES):
        xts = []
        for p in range(PPC):
            g = PPC * c + p
            bb, hh = g // H, g % H
            xts.append(np.ascontiguousarray(X[bb, 128 * hh:128 * (hh + 1), :].T))
        in_maps.append({
            "xt": np.stack(xts),
            "wq": wq_arr, "wk": wk_arr, "wv": wv_arr, "wo": wo_arr,
            "bias": bias_arr,
            "ones": np.ones((128, 16), dtype=np.float32),
        })
    return in_maps


_CACHED_NC = None


def get_nc():
    global _CACHED_NC
    if _CACHED_NC is None:
        _CACHED_NC = build_nc()
    return _CACHED_NC


def kernel(**inputs) -> np.ndarray:
    nc = get_nc()
    in_maps = _host_prep(**inputs)
    res = run_bass_kernel_spmd(nc, in_maps, list(range(NCORES)))
    out = np.empty((B, L, D), dtype=np.float32)
    for c in range(NCORES):
        y = res.results[c]["out"]  # (4, 128, 1024)
        for p in range(PPC):
            g = PPC * c + p
            bb, hh = g // H, g % H
            out[bb, 128 * hh:128 * (hh + 1), :] = y[p]
    return out



# revision 21
# speedup vs baseline: 3.0750x; 3.0750x over previous
"""Trainium2 Bass kernel for nn_MultiHeadAttention_66322884984909.

Math (faithful to reference):
  Q = X @ W_Q.T reshaped (B, H, L, hd) via DIRECT reshape -> head h owns rows
  128h:128(h+1) of the projected (L, D) matrix, reinterpreted as (2048, 64).
  The 32 (batch, head) pairs are independent: 8 cores x 4 pairs, no
  collectives.

Per pair (X_s = X[b, 128h:128h+128, :], (128, 1024)):
  Qh = (X_s @ W_Q.T).reshape(2048, 64);  Kh, Vh likewise
  S  = Qh @ Kh.T  (2048x2048, causal, NO 1/sqrt(hd) scale, no max-sub)
  O  = softmax(S) @ Vh  -> Y = O.reshape(128, 1024) @ W_O.T + b_O

Implementation notes:
  * Q^T/K^T come straight out of transposed projections (lhsT = W-block,
    rhs = X^T) into a t-major layout qht[d, 128t + j] = Qh[16j + t, d];
    strided matmul APs stream columns in natural l = 16j + t order so the
    causal block structure is unchanged. No DRAM bounce, no PE transposes.
  * V path is bf16 end to end (X, W_V, V, P=exp(S), O, W_O); Q/K/S stay
    fp32r. Measured on CPU: 3.8e-3 norm-rel error (tolerance 2e-2).
  * V natural projection -> bf16 nat tile (with embedded ones columns for
    the row-sum trick) -> one SBUF->SBUF scatter DMA into
    vh[kpos, 65*bb + d]. V for pairs 2,3 is emitted as deferred thunks
    drained inside group-0 attention (fills the Act-bound PE bubbles).
  * Normalization is deferred: PV psum is copied to SBUF so the psum
    accumulator frees immediately; recip/broadcast/mul run off the
    critical path. onorm holds O^T and O^T-shifted-by-one in the two
    partition halves so the output projection contracts 128 deep
    (8 accumulation steps), using natural bf16 W_O.T c-blocks.
  * Weights/X are loaded in per-128-row chunks into separate tiles so
    matmuls start as soon as their chunk lands.
  * Y(g) matmuls drain inside group g+1's attention.
"""

import numpy as np
import ml_dtypes

import concourse.bass as bass
from concourse import bacc
import concourse.mybir as mybir
import concourse.tile as tile
from concourse.bass_utils import run_bass_kernel_spmd

F32 = mybir.dt.float32
F32R = mybir.dt.float32r
BF16 = mybir.dt.bfloat16
FP16 = mybir.dt.float16
EXP = mybir.ActivationFunctionType.Exp

B, L, D = 2, 2048, 1024
H, HD = 16, 64
NCORES = 8
PPC = 4  # pairs per core
NG = PPC // 2  # pair groups


def build_nc(repeat=1):
    nc = bacc.Bacc(trn_type="TRN2", target_bir_lowering=False, debug=False)

    xth = nc.declare_dram_parameter("xth", [PPC, 128, 1024], FP16, isOutput=False)
    wq = nc.declare_dram_parameter("wq", [1024, 1024], FP16, isOutput=False)
    wk = nc.declare_dram_parameter("wk", [1024, 1024], FP16, isOutput=False)
    wv = nc.declare_dram_parameter("wv", [1024, 1024], FP16, isOutput=False)
    wo = nc.declare_dram_parameter("wo", [1024, 1024], BF16, isOutput=False)
    bias = nc.declare_dram_parameter("bias", [128, 1024], F32, isOutput=False)
    out = nc.declare_dram_parameter("out", [PPC, 128, 1024], F32, isOutput=True)
    vsh = nc.dram_tensor("vsh", [PPC, 128, 1040], BF16)

    with tile.TileContext(nc) as tc:
      for _rep in range(repeat):
        with (
            tc.tile_pool(name="consts", bufs=1) as consts,
            tc.tile_pool(name="wpool", bufs=1) as wpool,
            tc.tile_pool(name="qkp", bufs=1) as qkp,
            tc.tile_pool(name="vp", bufs=1) as vpool,
            tc.tile_pool(name="natp", bufs=2) as natp,
            tc.tile_pool(name="onp", bufs=1) as onp,
            tc.tile_pool(name="ptp", bufs=4) as ptp,
            tc.tile_pool(name="rp", bufs=2) as rp,
            tc.tile_pool(name="yp", bufs=2) as ypool,
        ):
            # ---- loads: X chunks per group, weights per-128-row chunk ----
            xthg = [consts.tile([128, 2, 8, 128], FP16, tag=f"xth{g}",
                                name=f"xthg{g}") for g in range(NG)]
            for g in range(NG):
                for i in range(2):
                    nc.scalar.dma_start(
                        out=xthg[g][:, i, :, :],
                        in_=xth[2 * g + i].rearrange("k (kb j) -> k kb j", j=128))

            wvt = [wpool.tile([128, 1024], FP16, tag=f"wv{kc}",
                              name=f"wvt{kc}") for kc in range(8)]
            wqt = [wpool.tile([128, 1024], FP16, tag=f"wq{kc}",
                              name=f"wqt{kc}") for kc in range(8)]
            wkt = [wpool.tile([128, 1024], FP16, tag=f"wk{kc}",
                              name=f"wkt{kc}") for kc in range(8)]
            for kc in range(8):
                nc.sync.dma_start(out=wvt[kc], in_=wv[128 * kc:128 * (kc + 1), :])
            for kc in range(8):
                nc.sync.dma_start(out=wqt[kc], in_=wq[128 * kc:128 * (kc + 1), :])
            for kc in range(8):
                nc.sync.dma_start(out=wkt[kc], in_=wk[128 * kc:128 * (kc + 1), :])

            bias_sb = consts.tile([128, 1024], F32)
            wo_sb = consts.tile([128, 8, 1024], BF16)

            qht = qkp.tile([128, NG, 2048], F32R, tag="qht", name="qht")
            kht = qkp.tile([128, NG, 2048], F32R, tag="kht", name="kht")
            vh = [vpool.tile([128, 16, 65], BF16, tag=f"vh{p}", name=f"vh{p}")
                  for p in range(PPC)]
            onorm = [onp.tile([128, 2048], BF16, tag=f"on{p}", name=f"on{p}")
                     for p in range(PPC)]
            # l-order columns viewed as (j, t): col l = 16j + t
            qv = qht.rearrange("p g (j t) -> p g j t", t=16)
            kv = kht.rearrange("p g (j t) -> p g j t", t=16)

            # preload the exp activation table during the projection phase
            scratch1 = consts.tile([1, 1], BF16, name="scr1")
            nc.scalar.activation(scratch1, xthg[0][0:1, 0, 0, 0:1], EXP)

            pps_ctx = tc.tile_pool(name="pps", bufs=4, space="PSUM")
            pps = pps_ctx.__enter__()

            # ------------- V projection pairs 0,1 + scatter ---------------
            def emit_vproj(i01, g, psum_pool, tag, psbufs):
                p = 2 * g + i01
                nat = natp.tile([128, 16, 65], BF16, tag="nat", bufs=2,
                                name=f"nat{p}")
                ops = []
                ops.append(lambda: nc.gpsimd.memset(nat[:, :, 64:65], 1.0))
                ps = psum_pool.tile([128, 1024], F32, tag=tag, bufs=psbufs,
                                    name=f"vps{p}")
                for jh in range(2):
                    for kh in range(2):
                        def mm4(jh=jh, kh=kh, ps=ps):
                            for kc in range(4 * kh, 4 * kh + 4):
                                nc.tensor.matmul(
                                    ps[:, jh * 512:(jh + 1) * 512],
                                    lhsT=xthg[g][:, i01, kc, :],
                                    rhs=wvt[kc][:, jh * 512:(jh + 1) * 512],
                                    start=(kc == 0), stop=(kc == 7),
                                )
                        ops.append(mm4)
                def fin(nat=nat, ps=ps, p=p):
                    nc.vector.tensor_copy(
                        nat[:, :, 0:64],
                        ps.rearrange("q (t e) -> q t e", e=64))
                    # bounce through DRAM: the write is a plain contiguous
                    # dump; the read's (rr t) partition dim has uniform
                    # stride 65 elems, so vh[16rr+t, 65bb+e] = nat[8bb+rr,t,e]
                    # in one 3-dim HWDGE DMA each way.
                    nc.scalar.dma_start(out=vsh[p], in_=nat)
                    nc.scalar.dma_start(
                        out=vh[p],
                        in_=vsh[p].rearrange(
                            "(bb rr) (t e) -> (rr t) bb e", bb=16, e=65))
                ops.append(fin)
                return ops

            for i01 in range(2):
                for op in emit_vproj(i01, 0, pps, "ps", 2):
                    op()

            # ------------- Q/K transposed projections ---------------------
            for wt, dstv in ((wqt, qv), (wkt, kv)):
                for c in range(8):
                    ps = pps.tile([128, 512], F32, tag="ps2", name="qkps")
                    # one psum accumulation chain open per bank at a time
                    for g in range(NG):
                        for i in range(2):
                            for kb in range(8):
                                nc.tensor.matmul(
                                    ps[:, 256 * g + 128 * i:
                                       256 * g + 128 * (i + 1)],
                                    lhsT=wt[kb][:, 128 * c:128 * (c + 1)],
                                    rhs=xthg[g][:, i, kb, :],
                                    start=(kb == 0), stop=(kb == 7),
                                )
                    psv = ps.rearrange("q (p j) -> q p j", p=PPC)
                    for t2 in range(2):
                        for h in range(2):
                            if t2 ^ h:
                                nc.vector.tensor_copy(
                                    dstv[64 * h:64 * (h + 1), :, :, 2 * c + t2],
                                    psv[64 * t2:64 * t2 + 64, h::2, :])
                            else:
                                nc.scalar.copy(
                                    dstv[64 * h:64 * (h + 1), :, :, 2 * c + t2],
                                    psv[64 * t2:64 * t2 + 64, h::2, :])

            pps_ctx.__exit__(None, None, None)

            nc.sync.dma_start(out=wo_sb,
                              in_=wo.rearrange("(c k) j -> k c j", k=128))
            nc.sync.dma_start(out=bias_sb, in_=bias[:])

            # ------------- attention + output projection ------------------
            pending = []

            def drain(n):
                for _ in range(min(n, len(pending))):
                    pending.pop(0)()

            with tc.tile_pool(name="aps", bufs=1, space="PSUM") as aps:

                pvs_by_a = {}

                def emit_attention(g):
                    # software pipeline: PV for iteration k is emitted during
                    # iteration k+1, so the PE streams S matmuls without
                    # blocking on exp, and Act stays saturated.
                    iters = [(a, gg, i)
                             for a in range(4)
                             for gg in range(2 * a + 2)
                             for i in range(2)]

                    def emit_s(a, gg, i):
                        # Diagonal blocks (gg >= 2a): q2-half r = 2(gg-2a)+q2
                        # only has visible columns j >= 128r of the chunk.
                        # S computes cols >= min(128r, 256) (fp32r needs
                        # ap>=256), exp covers cols >= 128r, and the causal
                        # boundary is a single 128-wide triangular band
                        # handled by a small affine_select.
                        diag = gg >= 2 * a
                        st = aps.tile([128, 1024], F32, tag="sts",
                                      bufs=2, name="sts")
                        for q2 in range(2):
                            bb = 2 * gg + q2
                            r = 2 * (gg - 2 * a) + q2
                            c0 = min(128 * r, 256) if diag else 0
                            nc.tensor.matmul(
                                st[:, q2 * 512 + c0:(q2 + 1) * 512],
                                lhsT=kht[64 * i:64 * (i + 1), g,
                                         128 * bb:128 * (bb + 1)],
                                rhs=qht[64 * i:64 * (i + 1), g,
                                        512 * a + c0:512 * (a + 1)],
                                start=True, stop=True,
                            )
                        pt = ptp.tile([128, 1024], BF16, tag="pt", name="pt")
                        if not diag:
                            nc.scalar.activation(pt, st, EXP)
                        else:
                            for q2 in range(2):
                                r = 2 * (gg - 2 * a) + q2
                                e0 = q2 * 512 + 128 * r
                                nc.scalar.activation(
                                    pt[:, e0:(q2 + 1) * 512],
                                    st[:, e0:(q2 + 1) * 512], EXP)
                                # triangular boundary band: keep iff u >= q
                                nc.gpsimd.affine_select(
                                    out=pt[:, e0:e0 + 128],
                                    in_=pt[:, e0:e0 + 128],
                                    compare_op=mybir.AluOpType.is_ge,
                                    fill=0.0,
                                    base=0,
                                    pattern=[[1, 128]],
                                    channel_multiplier=-1,
                                )
                        return pt

                    def emit_pv(a, gg, i, pt):
                        diag = gg >= 2 * a
                        if gg == 0:
                            pvs_by_a[(g, a, i)] = aps.tile(
                                [65, 512], F32, tag=f"pv{i}", name=f"pv{i}")
                        pv = pvs_by_a[(g, a, i)]
                        for q2 in range(2):
                            bb = 2 * gg + q2
                            r = 2 * (gg - 2 * a) + q2
                            c0 = 128 * r if diag else 0
                            nc.tensor.matmul(
                                pv[:, c0:512],
                                lhsT=vh[2 * g + i][:, bb, :],
                                rhs=pt[:, q2 * 512 + c0:(q2 + 1) * 512],
                                start=(bb == 0), stop=(bb == 4 * a + 3),
                            )
                        if gg == 2 * a + 1:
                            # free the psum accumulator fast; finish
                            # normalization off the critical path
                            p = 2 * g + i
                            osb = rp.tile([65, 512], F32, tag="osb",
                                          bufs=2, name=f"osb{p}_{a}")
                            nc.vector.tensor_copy(osb, pv)
                            r1 = rp.tile([1, 512], F32, tag="r1", name="r1")
                            nc.vector.reciprocal(r1, osb[64:65, :])
                            rb = rp.tile([64, 512], F32, tag="rb", name="rb")
                            nc.gpsimd.partition_broadcast(rb, r1)
                            osv = osb.rearrange(
                                "q (r c s2) -> q c s2 r", c=8, s2=2)
                            rbv = rb.rearrange(
                                "q (r c s2) -> q c s2 r", c=8, s2=2)
                            onv = onorm[p].rearrange(
                                "q (c s2 r) -> q c s2 r", c=8, s2=2)
                            nc.vector.tensor_mul(
                                onv[0:64, :, 0, 32 * a:32 * (a + 1)],
                                osv[0:64, :, 0, :], rbv[:, :, 0, :])
                            nc.vector.tensor_mul(
                                onv[64:128, :, 0, 32 * a:32 * (a + 1)],
                                osv[0:64, :, 1, :], rbv[:, :, 1, :])

                    prev = None
                    for it in iters:
                        pt = emit_s(*it)
                        if prev is not None:
                            emit_pv(*prev[0], prev[1])
                        prev = (it, pt)
                        drain(1)
                    emit_pv(*prev[0], prev[1])

                def emit_y(g):
                    for i in range(2):
                        p = 2 * g + i
                        onp_ = onorm[p]
                        yps = aps.tile([128, 1024], F32, tag="aux",
                                       name=f"yps{p}")

                        def steps(c0, yps=yps, onp_=onp_):
                            for c in range(c0, c0 + 2):
                                for jo in range(2):
                                    nc.tensor.matmul(
                                        yps[:, 512 * jo:512 * (jo + 1)],
                                        lhsT=onp_[:, 256 * c:256 * c + 128],
                                        rhs=wo_sb[:, c,
                                                  512 * jo:512 * (jo + 1)],
                                        start=(c == 0), stop=(c == 7),
                                    )

                        def fin(p=p, yps=yps):
                            ysb = ypool.tile([128, 1024], F32, tag="ysb",
                                             name=f"ysb{p}")
                            nc.vector.tensor_add(ysb, yps, bias_sb)
                            nc.sync.dma_start(out=out[p], in_=ysb)

                        for c0 in range(0, 8, 2):
                            pending.append(lambda c0=c0, steps=steps: steps(c0))
                        pending.append(fin)

                # V for pairs 2,3 drains inside group-0 attention (aux psum)
                for i01 in range(2):
                    pending.extend(emit_vproj(i01, 1, aps, "aux", 1))

                for g in range(NG):
                    emit_attention(g)
                    emit_y(g)
                drain(len(pending))

    nc.finalize()
    return nc


def _host_prep(input_seq_embs, W_Q, W_K, W_V, W_O, b_O):
    X = np.asarray(input_seq_embs, dtype=np.float32)
    wq_arr = np.ascontiguousarray(np.asarray(W_Q, dtype=np.float32).T).astype(np.float16)
    wk_arr = np.ascontiguousarray(np.asarray(W_K, dtype=np.float32).T).astype(np.float16)
    wv_arr = np.ascontiguousarray(np.asarray(W_V, dtype=np.float32).T).astype(np.float16)
    wo_arr = np.ascontiguousarray(
        np.asarray(W_O, dtype=np.float32).T).astype(ml_dtypes.bfloat16)
    bias_arr = np.ascontiguousarray(
        np.broadcast_to(np.asarray(b_O, dtype=np.float32), (128, 1024)))

    in_maps = []
    for c in range(NCORES):
        xts = []
        for p in range(PPC):
            g = PPC * c + p
            bb, hh = g // H, g % H
            # xth[p][k, 8*kb + j-block]: per-partition contiguous chunks of
            # X_s^T: xth[p][k, 128*kb + j] = X_s^T[128*kb + k, j]
            xs_t = X[bb, 128 * hh:128 * (hh + 1), :].T  # (1024, 128)
            xts.append(np.ascontiguousarray(
                xs_t.reshape(8, 128, 128).transpose(1, 0, 2).reshape(128, 1024)
            ).astype(np.float16))
        in_maps.append({
            "xth": np.stack(xts),
            "wq": wq_arr, "wk": wk_arr, "wv": wv_arr, "wo": wo_arr,
            "bias": bias_arr,
        })
    return in_maps


_CACHED_NC = None


def get_nc():
    global _CACHED_NC
    if _CACHED_NC is None:
        _CACHED_NC = build_nc()
    return _CACHED_NC


def kernel(**inputs) -> np.ndarray:
    nc = get_nc()
    in_maps = _host_prep(**inputs)
    res = run_bass_kernel_spmd(nc, in_maps, list(range(NCORES)))
    out = np.empty((B, L, D), dtype=np.float32)
    for c in range(NCORES):
        y = res.results[c]["out"]  # (4, 128, 1024)
        for p in range(PPC):
            g = PPC * c + p
            bb, hh = g // H, g % H
            out[bb, 128 * hh:128 * (hh + 1), :] = y[p]
    return out


# revision 30
# speedup vs baseline: 4.4742x; 1.4550x over previous
"""Trainium2 Bass kernel for nn_MultiHeadAttention_66322884984909.

Math (faithful to reference):
  Q = X @ W_Q.T reshaped (B, H, L, hd) via DIRECT reshape -> head h owns rows
  128h:128(h+1) of the projected (L, D) matrix, reinterpreted as (2048, 64).
  The 32 (batch, head) pairs are independent: 8 cores x 4 pairs, no
  collectives.

Per pair (X_s = X[b, 128h:128h+128, :], (128, 1024)):
  Qh = (X_s @ W_Q.T).reshape(2048, 64);  Kh, Vh likewise
  S  = Qh @ Kh.T  (2048x2048, causal, NO 1/sqrt(hd) scale, no max-sub)
  O  = softmax(S) @ Vh  -> Y = O.reshape(128, 1024) @ W_O.T + b_O

Implementation notes:
  * Q^T/K^T come straight out of transposed projections (lhsT = W-block,
    rhs = X^T) into a t-major layout qht[d, 128t + j] = Qh[16j + t, d];
    strided matmul APs stream columns in natural l = 16j + t order so the
    causal block structure is unchanged. No DRAM bounce, no PE transposes.
  * V path is bf16 end to end (X, W_V, V, P=exp(S), O, W_O); Q/K/S stay
    fp32r. Measured on CPU: 3.8e-3 norm-rel error (tolerance 2e-2).
  * V natural projection -> bf16 nat tile (with embedded ones columns for
    the row-sum trick) -> one SBUF->SBUF scatter DMA into
    vh[kpos, 65*bb + d]. V for pairs 2,3 is emitted as deferred thunks
    drained inside group-0 attention (fills the Act-bound PE bubbles).
  * Normalization is deferred: PV psum is copied to SBUF so the psum
    accumulator frees immediately; recip/broadcast/mul run off the
    critical path. onorm holds O^T and O^T-shifted-by-one in the two
    partition halves so the output projection contracts 128 deep
    (8 accumulation steps), using natural bf16 W_O.T c-blocks.
  * Weights/X are loaded in per-128-row chunks into separate tiles so
    matmuls start as soon as their chunk lands.
  * Y(g) matmuls drain inside group g+1's attention.
"""

import numpy as np
import ml_dtypes

import concourse.bass as bass
from concourse import bacc
import concourse.mybir as mybir
import concourse.tile as tile
from concourse.bass_utils import run_bass_kernel_spmd

F32 = mybir.dt.float32
F32R = mybir.dt.float32r
BF16 = mybir.dt.bfloat16
FP16 = mybir.dt.float16
EXP = mybir.ActivationFunctionType.Exp

B, L, D = 2, 2048, 1024
H, HD = 16, 64
NCORES = 8
PPC = 4  # pairs per core
NG = PPC // 2  # pair groups


def build_nc(repeat=1):
    nc = bacc.Bacc(trn_type="TRN2", target_bir_lowering=False, debug=False)

    xth = nc.declare_dram_parameter("xth", [PPC, 128, 1024], FP16, isOutput=False)
    wq = nc.declare_dram_parameter("wq", [1024, 1024], FP16, isOutput=False)
    wk = nc.declare_dram_parameter("wk", [1024, 1024], FP16, isOutput=False)
    wv = nc.declare_dram_parameter("wv", [1024, 1024], FP16, isOutput=False)
    wo = nc.declare_dram_parameter("wo", [1024, 1024], BF16, isOutput=False)
    bias = nc.declare_dram_parameter("bias", [128, 1024], F32, isOutput=False)
    out = nc.declare_dram_parameter("out", [PPC, 128, 1024], F32, isOutput=True)
    vsh = nc.dram_tensor("vsh", [PPC, 128, 1040], BF16)

    with tile.TileContext(nc) as tc:
      for _rep in range(repeat):
        with (
            tc.tile_pool(name="consts", bufs=1) as consts,
            tc.tile_pool(name="wpool", bufs=1) as wpool,
            tc.tile_pool(name="qkp", bufs=1) as qkp,
            tc.tile_pool(name="vp", bufs=1) as vpool,
            tc.tile_pool(name="natp", bufs=2) as natp,
            tc.tile_pool(name="onp", bufs=1) as onp,
            tc.tile_pool(name="ptp", bufs=6) as ptp,
            tc.tile_pool(name="rp", bufs=2) as rp,
            tc.tile_pool(name="yp", bufs=2) as ypool,
        ):
            # ---- loads: X chunks per group, weights per-128-row chunk ----
            xthg = [consts.tile([128, 2, 8, 128], FP16, tag=f"xth{g}",
                                name=f"xthg{g}") for g in range(NG)]
            for g in range(NG):
                for i in range(2):
                    nc.scalar.dma_start(
                        out=xthg[g][:, i, :, :],
                        in_=xth[2 * g + i].rearrange("k (kb j) -> k kb j", j=128))

            wvt = [wpool.tile([128, 1024], FP16, tag=f"wv{kc}",
                              name=f"wvt{kc}") for kc in range(8)]
            wqt = [wpool.tile([128, 1024], FP16, tag=f"wq{kc}",
                              name=f"wqt{kc}") for kc in range(8)]
            wkt = [wpool.tile([128, 1024], FP16, tag=f"wk{kc}",
                              name=f"wkt{kc}") for kc in range(8)]
            for kc in range(8):
                nc.sync.dma_start(out=wvt[kc], in_=wv[128 * kc:128 * (kc + 1), :])
            for kc in range(8):
                nc.sync.dma_start(out=wqt[kc], in_=wq[128 * kc:128 * (kc + 1), :])
            for kc in range(8):
                nc.sync.dma_start(out=wkt[kc], in_=wk[128 * kc:128 * (kc + 1), :])

            bias_sb = consts.tile([128, 1024], F32)
            wo_sb = consts.tile([128, 8, 1024], BF16)

            qht = qkp.tile([128, NG, 2048], F32R, tag="qht", name="qht")
            kht = qkp.tile([128, NG, 2048], F32R, tag="kht", name="kht")
            vh = [vpool.tile([128, 16, 65], BF16, tag=f"vh{p}", name=f"vh{p}")
                  for p in range(PPC)]
            onorm = [onp.tile([128, 2048], BF16, tag=f"on{p}", name=f"on{p}")
                     for p in range(PPC)]
            # l-order columns viewed as (j, t): col l = 16j + t
            qv = qht.rearrange("p g (j t) -> p g j t", t=16)
            kv = kht.rearrange("p g (j t) -> p g j t", t=16)

            # preload the exp activation table during the projection phase
            scratch1 = consts.tile([1, 1], BF16, name="scr1")
            nc.scalar.activation(scratch1, xthg[0][0:1, 0, 0, 0:1], EXP)

            pps_ctx = tc.tile_pool(name="pps", bufs=4, space="PSUM")
            pps = pps_ctx.__enter__()

            # ------------- V projection + scatter -------------------------
            def emit_vproj(i01, g, psum_pool, tag, psbufs):
                p = 2 * g + i01
                nat = natp.tile([128, 16, 65], BF16, tag="nat", bufs=2,
                                name=f"nat{p}")
                ops = []
                ops.append(lambda: nc.gpsimd.memset(nat[:, :, 64:65], 1.0))
                for jh in range(2):
                    ps = psum_pool.tile([128, 512], F32, tag=tag, bufs=psbufs,
                                        name=f"vps{p}_{jh}")

                    def mm8(jh=jh, ps=ps):
                        for kc in range(8):
                            nc.tensor.matmul(
                                ps,
                                lhsT=xthg[g][:, i01, kc, :],
                                rhs=wvt[kc][:, jh * 512:(jh + 1) * 512],
                                start=(kc == 0), stop=(kc == 7),
                            )
                    ops.append(mm8)

                    def cp(jh=jh, ps=ps):
                        nc.vector.tensor_copy(
                            nat[:, 8 * jh:8 * (jh + 1), 0:64],
                            ps.rearrange("q (t e) -> q t e", e=64))
                    ops.append(cp)

                def fin(nat=nat, p=p):
                    # bounce through DRAM: the write is a plain contiguous
                    # dump; the read's (rr t) partition dim has uniform
                    # stride 65 elems, so vh[16rr+t, 65bb+e] = nat[8bb+rr,t,e]
                    # in one 3-dim HWDGE DMA each way. On the sync queue these
                    # sit AFTER the weight-chunk loads, so they don't steal
                    # DMA bandwidth from the projection-critical stream.
                    nc.sync.dma_start(out=vsh[p], in_=nat)
                    nc.sync.dma_start(
                        out=vh[p],
                        in_=vsh[p].rearrange(
                            "(bb rr) (t e) -> (rr t) bb e", bb=16, e=65))
                ops.append(fin)
                return ops

            for i01 in range(2):
                for op in emit_vproj(i01, 0, pps, "ps2", 4):
                    op()

            # ------------- Q/K transposed projections (per group) ---------
            def emit_qkproj_cblock(wt, dstv, g, c, psum_pool, tag, psbufs):
                """Returns thunks: matmul chains + psum->qht strided copies."""
                ps = psum_pool.tile([128, 512], F32, tag=tag, bufs=psbufs,
                                    name=f"qkps{g}{c}")
                ops = []

                def chain(i, ps=ps):
                    # one psum accumulation chain open per bank at a time
                    for kb in range(8):
                        nc.tensor.matmul(
                            ps[:, 128 * i:128 * (i + 1)],
                            lhsT=wt[kb][:, 128 * c:128 * (c + 1)],
                            rhs=xthg[g][:, i, kb, :],
                            start=(kb == 0), stop=(kb == 7),
                        )
                ops.append(lambda: chain(0))
                ops.append(lambda: chain(1))

                def copies(ps=ps):
                    psv = ps[:, 0:256].rearrange("q (i j) -> q i j", i=2)
                    for t2 in range(2):
                        for i in range(2):
                            dst = dstv[64 * i:64 * (i + 1), g, :, 2 * c + t2]
                            src = psv[64 * t2:64 * t2 + 64, i, :]
                            # g0 runs in the projection phase where Act is
                            # idle; g1 copies drain inside attention where
                            # Act is exp-bound, so they stay on DVE.
                            if g == 0 and not (t2 ^ i):
                                nc.scalar.copy(dst, src)
                            else:
                                nc.vector.tensor_copy(dst, src)
                ops.append(copies)
                return ops

            for wt, dstv in ((wqt, qv), (wkt, kv)):
                for c in range(8):
                    for op in emit_qkproj_cblock(wt, dstv, 0, c, pps,
                                                 "ps2", 4):
                        op()

            pps_ctx.__exit__(None, None, None)

            nc.sync.dma_start(out=wo_sb,
                              in_=wo.rearrange("(c k) j -> k c j", k=128))
            nc.sync.dma_start(out=bias_sb, in_=bias[:])

            # ------------- attention + output projection ------------------
            pending = []

            def drain(n):
                for _ in range(min(n, len(pending))):
                    pending.pop(0)()

            with tc.tile_pool(name="aps", bufs=1, space="PSUM") as aps:

                pvs_by_a = {}

                def emit_attention(g):
                    # software pipeline: PV for iteration k is emitted during
                    # iteration k+1, so the PE streams S matmuls without
                    # blocking on exp, and Act stays saturated.
                    iters = [(a, gg, i)
                             for a in range(4)
                             for gg in range(2 * a + 2)
                             for i in range(2)]

                    def emit_s(a, gg, i):
                        # Diagonal blocks (gg >= 2a): q2-half r = 2(gg-2a)+q2
                        # only has visible columns j >= 128r of the chunk.
                        # S computes cols >= min(128r, 256) (fp32r needs
                        # ap>=256), exp covers cols >= 128r, and the causal
                        # boundary is a single 128-wide triangular band
                        # handled by a small affine_select.
                        diag = gg >= 2 * a
                        st = aps.tile([128, 1024], F32, tag="sts",
                                      bufs=2, name="sts")
                        for q2 in range(2):
                            bb = 2 * gg + q2
                            r = 2 * (gg - 2 * a) + q2
                            c0 = min(128 * r, 256) if diag else 0
                            nc.tensor.matmul(
                                st[:, q2 * 512 + c0:(q2 + 1) * 512],
                                lhsT=kht[64 * i:64 * (i + 1), g,
                                         128 * bb:128 * (bb + 1)],
                                rhs=qht[64 * i:64 * (i + 1), g,
                                        512 * a + c0:512 * (a + 1)],
                                start=True, stop=True,
                            )
                        pt = ptp.tile([128, 1024], BF16, tag="pt", name="pt")
                        if not diag:
                            nc.scalar.activation(pt, st, EXP)
                        else:
                            for q2 in range(2):
                                r = 2 * (gg - 2 * a) + q2
                                e0 = q2 * 512 + 128 * r
                                nc.scalar.activation(
                                    pt[:, e0:(q2 + 1) * 512],
                                    st[:, e0:(q2 + 1) * 512], EXP)
                                # triangular boundary band: keep iff u >= q
                                nc.gpsimd.affine_select(
                                    out=pt[:, e0:e0 + 128],
                                    in_=pt[:, e0:e0 + 128],
                                    compare_op=mybir.AluOpType.is_ge,
                                    fill=0.0,
                                    base=0,
                                    pattern=[[1, 128]],
                                    channel_multiplier=-1,
                                )
                        return pt

                    def emit_pv(a, gg, i, pt):
                        diag = gg >= 2 * a
                        if gg == 0:
                            pvs_by_a[(g, a, i)] = aps.tile(
                                [65, 512], F32, tag=f"pv{i}", name=f"pv{i}")
                        pv = pvs_by_a[(g, a, i)]
                        for q2 in range(2):
                            bb = 2 * gg + q2
                            r = 2 * (gg - 2 * a) + q2
                            c0 = 128 * r if diag else 0
                            nc.tensor.matmul(
                                pv[:, c0:512],
                                lhsT=vh[2 * g + i][:, bb, :],
                                rhs=pt[:, q2 * 512 + c0:(q2 + 1) * 512],
                                start=(bb == 0), stop=(bb == 4 * a + 3),
                            )
                        if gg == 2 * a + 1:
                            # free the psum accumulator fast; finish
                            # normalization off the critical path
                            p = 2 * g + i
                            osb = rp.tile([65, 512], F32, tag="osb",
                                          bufs=2, name=f"osb{p}_{a}")
                            nc.vector.tensor_copy(osb, pv)
                            r1 = rp.tile([1, 512], F32, tag="r1", name="r1")
                            nc.vector.reciprocal(r1, osb[64:65, :])
                            rb = rp.tile([64, 512], F32, tag="rb", name="rb")
                            nc.gpsimd.partition_broadcast(rb, r1)
                            osv = osb.rearrange(
                                "q (r c s2) -> q c s2 r", c=8, s2=2)
                            rbv = rb.rearrange(
                                "q (r c s2) -> q c s2 r", c=8, s2=2)
                            onv = onorm[p].rearrange(
                                "q (c s2 r) -> q c s2 r", c=8, s2=2)
                            nc.vector.tensor_mul(
                                onv[0:64, :, 0, 32 * a:32 * (a + 1)],
                                osv[0:64, :, 0, :], rbv[:, :, 0, :])
                            nc.vector.tensor_mul(
                                onv[64:128, :, 0, 32 * a:32 * (a + 1)],
                                osv[0:64, :, 1, :], rbv[:, :, 1, :])

                    prev = None
                    for it in iters:
                        pt = emit_s(*it)
                        if prev is not None:
                            emit_pv(*prev[0], prev[1])
                        prev = (it, pt)
                        drain(2)
                    emit_pv(*prev[0], prev[1])

                def emit_y(g, inline=False):
                    for i in range(2):
                        p = 2 * g + i
                        onp_ = onorm[p]
                        ysb = ypool.tile([128, 1024], F32, tag="ysb",
                                         name=f"ysb{p}")
                        for jo in range(2):
                            yps = aps.tile([128, 512], F32, tag="aux",
                                           bufs=2, name=f"yps{p}_{jo}")

                            def mm(jo=jo, yps=yps, onp_=onp_):
                                for c in range(8):
                                    nc.tensor.matmul(
                                        yps,
                                        lhsT=onp_[:, 256 * c:256 * c + 128],
                                        rhs=wo_sb[:, c,
                                                  512 * jo:512 * (jo + 1)],
                                        start=(c == 0), stop=(c == 7),
                                    )

                            def fin(jo=jo, yps=yps, ysb=ysb, p=p):
                                nc.vector.tensor_add(
                                    ysb[:, 512 * jo:512 * (jo + 1)], yps,
                                    bias_sb[:, 512 * jo:512 * (jo + 1)])
                                if jo == 1:
                                    nc.sync.dma_start(out=out[p], in_=ysb)

                            if inline:
                                mm(); fin()
                            else:
                                pending.append(mm)
                                pending.append(fin)

                # group-1 Q/K projections + V pairs 2,3 drain inside
                # group-0 attention (aux psum, 1 bank each)
                for wt, dstv in ((wqt, qv), (wkt, kv)):
                    for c in range(8):
                        pending.extend(emit_qkproj_cblock(
                            wt, dstv, 1, c, aps, "aux", 2))
                for i01 in range(2):
                    pending.extend(emit_vproj(i01, 1, aps, "aux", 2))

                emit_attention(0)
                emit_y(0)
                emit_attention(1)
                emit_y(1, inline=True)
                drain(len(pending))

    nc.finalize()
    return nc


def _host_prep(input_seq_embs, W_Q, W_K, W_V, W_O, b_O):
    X = np.asarray(input_seq_embs, dtype=np.float32)
    wq_arr = np.ascontiguousarray(np.asarray(W_Q, dtype=np.float32).T).astype(np.float16)
    wk_arr = np.ascontiguousarray(np.asarray(W_K, dtype=np.float32).T).astype(np.float16)
    wv_arr = np.ascontiguousarray(np.asarray(W_V, dtype=np.float32).T).astype(np.float16)
    wo_arr = np.ascontiguousarray(
        np.asarray(W_O, dtype=np.float32).T).astype(ml_dtypes.bfloat16)
    bias_arr = np.ascontiguousarray(
        np.broadcast_to(np.asarray(b_O, dtype=np.float32), (128, 1024)))

    in_maps = []
    for c in range(NCORES):
        xts = []
        for p in range(PPC):
            g = PPC * c + p
            bb, hh = g // H, g % H
            # xth[p][k, 8*kb + j-block]: per-partition contiguous chunks of
            # X_s^T: xth[p][k, 128*kb + j] = X_s^T[128*kb + k, j]
            xs_t = X[bb, 128 * hh:128 * (hh + 1), :].T  # (1024, 128)
            xts.append(np.ascontiguousarray(
                xs_t.reshape(8, 128, 128).transpose(1, 0, 2).reshape(128, 1024)
            ).astype(np.float16))
        in_maps.append({
            "xth": np.stack(xts),
            "wq": wq_arr, "wk": wk_arr, "wv": wv_arr, "wo": wo_arr,
            "bias": bias_arr,
        })
    return in_maps


_CACHED_NC = None


def get_nc():
    global _CACHED_NC
    if _CACHED_NC is None:
        _CACHED_NC = build_nc()
    return _CACHED_NC


def kernel(**inputs) -> np.ndarray:
    nc = get_nc()
    in_maps = _host_prep(**inputs)
    res = run_bass_kernel_spmd(nc, in_maps, list(range(NCORES)))
    out = np.empty((B, L, D), dtype=np.float32)
    for c in range(NCORES):
        y = res.results[c]["out"]  # (4, 128, 1024)
        for p in range(PPC):
            g = PPC * c + p
            bb, hh = g // H, g % H
            out[bb, 128 * hh:128 * (hh + 1), :] = y[p]
    return out


# revision 32
# speedup vs baseline: 5.6674x; 1.2667x over previous
"""Trainium2 Bass kernel for nn_MultiHeadAttention_66322884984909.

Math (faithful to reference):
  Q = X @ W_Q.T reshaped (B, H, L, hd) via DIRECT reshape -> head h owns rows
  128h:128(h+1) of the projected (L, D) matrix, reinterpreted as (2048, 64).
  The 32 (batch, head) pairs are independent: 8 cores x 4 pairs, no
  collectives.

Per pair (X_s = X[b, 128h:128h+128, :], (128, 1024)):
  Qh = (X_s @ W_Q.T).reshape(2048, 64);  Kh, Vh likewise
  S  = Qh @ Kh.T  (2048x2048, causal, NO 1/sqrt(hd) scale, no max-sub)
  O  = softmax(S) @ Vh  -> Y = O.reshape(128, 1024) @ W_O.T + b_O

Implementation notes:
  * Q^T/K^T come straight out of transposed projections (lhsT = W-chunk,
    rhs = X^T, both fp16) written by strided psum->SBUF copies into the
    natural l-order layout qht[d, l] = Qh[l, d]. No PE transposes.
  * Projections are fp16 (X, W_Q/K/V); V/P/O/W_O are bf16; S stays fp32r.
    Measured: 3.2e-3 norm-rel error on HW (tolerance 2e-2).
  * V natural projection -> bf16 nat tile (with embedded ones columns for
    the row-sum trick) -> DRAM bounce: contiguous dump, then a single
    3-dim HWDGE read whose (rr t) partition dim has uniform stride, which
    lands vh[kpos, 65*bb + e] = nat[8*bb + rr, t, e].
  * Attention is software-pipelined: PV for iteration k is emitted during
    iteration k+1 so the PE streams S matmuls while Act (exp) runs; exp
    is the attention-phase bottleneck, so group-1 Q/K/V projections and
    group-0 output projections are deferred thunks drained into the
    Act-bound bubbles of the preceding group's attention.
  * Diagonal S blocks are restricted to their visible columns: S and PV
    skip fully-masked column ranges, exp covers only visible columns, and
    the causal boundary is a 128-wide triangular band zeroed by a small
    affine_select (per-half), keeping Pool off the critical path.
  * Normalization is deferred: PV psum is copied to SBUF so the psum
    accumulator frees immediately; recip/broadcast/mul run off the
    critical path. onorm is t-major with O^T and O^T-shifted-by-one in
    the two partition halves, so the output projection contracts 128
    deep in 8 steps with contiguous stationary APs and natural bf16
    W_O.T c-blocks.
  * Weights/X are loaded in per-128-row chunks into separate tiles so
    matmuls start as soon as their chunk lands; V bounces ride the sync
    queue behind the weight stream.
  * PSUM: 2x[128,1024] S tiles + 2 PV accumulators + 2x[128,512] aux
    (projection drains / output projection halves) = exactly 8 banks,
    one accumulation chain per bank.
"""

import numpy as np
import ml_dtypes

import concourse.bass as bass
from concourse import bacc
import concourse.mybir as mybir
import concourse.tile as tile
from concourse.bass_utils import run_bass_kernel_spmd

F32 = mybir.dt.float32
F32R = mybir.dt.float32r
BF16 = mybir.dt.bfloat16
FP16 = mybir.dt.float16
EXP = mybir.ActivationFunctionType.Exp

B, L, D = 2, 2048, 1024
H, HD = 16, 64
NCORES = 8
PPC = 4  # pairs per core
NG = PPC // 2  # pair groups


def build_nc(repeat=1):
    nc = bacc.Bacc(trn_type="TRN2", target_bir_lowering=False, debug=False)

    xth = nc.declare_dram_parameter("xth", [PPC, 128, 1024], FP16, isOutput=False)
    wq = nc.declare_dram_parameter("wq", [1024, 1024], FP16, isOutput=False)
    wk = nc.declare_dram_parameter("wk", [1024, 1024], FP16, isOutput=False)
    wv = nc.declare_dram_parameter("wv", [1024, 1024], FP16, isOutput=False)
    wo = nc.declare_dram_parameter("wo", [1024, 1024], BF16, isOutput=False)
    bias = nc.declare_dram_parameter("bias", [128, 1024], F32, isOutput=False)
    out = nc.declare_dram_parameter("out", [PPC, 128, 1024], F32, isOutput=True)
    vsh = nc.dram_tensor("vsh", [PPC, 128, 1040], BF16)

    with tile.TileContext(nc) as tc:
      for _rep in range(repeat):
        with (
            tc.tile_pool(name="consts", bufs=1) as consts,
            tc.tile_pool(name="wpool", bufs=1) as wpool,
            tc.tile_pool(name="qkp", bufs=1) as qkp,
            tc.tile_pool(name="vp", bufs=1) as vpool,
            tc.tile_pool(name="natp", bufs=2) as natp,
            tc.tile_pool(name="onp", bufs=1) as onp,
            tc.tile_pool(name="ptp", bufs=6) as ptp,
            tc.tile_pool(name="rp", bufs=2) as rp,
            tc.tile_pool(name="yp", bufs=2) as ypool,
        ):
            # ---- loads: X chunks per group, weights per-128-row chunk ----
            xthg = [consts.tile([128, 2, 8, 128], FP16, tag=f"xth{g}",
                                name=f"xthg{g}") for g in range(NG)]
            for g in range(NG):
                for i in range(2):
                    nc.scalar.dma_start(
                        out=xthg[g][:, i, :, :],
                        in_=xth[2 * g + i].rearrange("k (kb j) -> k kb j", j=128))

            wvt = [wpool.tile([128, 1024], FP16, tag=f"wv{kc}",
                              name=f"wvt{kc}") for kc in range(8)]
            wqt = [wpool.tile([128, 1024], FP16, tag=f"wq{kc}",
                              name=f"wqt{kc}") for kc in range(8)]
            wkt = [wpool.tile([128, 1024], FP16, tag=f"wk{kc}",
                              name=f"wkt{kc}") for kc in range(8)]
            for kc in range(8):
                nc.sync.dma_start(out=wvt[kc], in_=wv[128 * kc:128 * (kc + 1), :])
            for kc in range(8):
                nc.sync.dma_start(out=wqt[kc], in_=wq[128 * kc:128 * (kc + 1), :])
            for kc in range(8):
                nc.sync.dma_start(out=wkt[kc], in_=wk[128 * kc:128 * (kc + 1), :])

            bias_sb = consts.tile([128, 1024], F32)
            wo_sb = consts.tile([128, 8, 1024], BF16)

            qht = qkp.tile([128, NG, 2048], F32R, tag="qht", name="qht")
            kht = qkp.tile([128, NG, 2048], F32R, tag="kht", name="kht")
            vh = [vpool.tile([128, 16, 65], BF16, tag=f"vh{p}", name=f"vh{p}")
                  for p in range(PPC)]
            onorm = [onp.tile([128, 2048], BF16, tag=f"on{p}", name=f"on{p}")
                     for p in range(PPC)]
            # l-order columns viewed as (j, t): col l = 16j + t
            qv = qht.rearrange("p g (j t) -> p g j t", t=16)
            kv = kht.rearrange("p g (j t) -> p g j t", t=16)

            # preload the exp activation table during the projection phase
            scratch1 = consts.tile([1, 1], BF16, name="scr1")
            nc.scalar.activation(scratch1, xthg[0][0:1, 0, 0, 0:1], EXP)

            pps_ctx = tc.tile_pool(name="pps", bufs=4, space="PSUM")
            pps = pps_ctx.__enter__()

            # ------------- V projection + scatter -------------------------
            def emit_vproj(i01, g, psum_pool, tag, psbufs):
                p = 2 * g + i01
                nat = natp.tile([128, 16, 65], BF16, tag="nat", bufs=2,
                                name=f"nat{p}")
                ops = []
                ops.append(lambda: nc.gpsimd.memset(nat[:, :, 64:65], 1.0))
                for jh in range(2):
                    ps = psum_pool.tile([128, 512], F32, tag=tag, bufs=psbufs,
                                        name=f"vps{p}_{jh}")

                    def mm8(jh=jh, ps=ps):
                        for kc in range(8):
                            nc.tensor.matmul(
                                ps,
                                lhsT=xthg[g][:, i01, kc, :],
                                rhs=wvt[kc][:, jh * 512:(jh + 1) * 512],
                                start=(kc == 0), stop=(kc == 7),
                            )
                    ops.append(mm8)

                    def cp(jh=jh, ps=ps):
                        nc.vector.tensor_copy(
                            nat[:, 8 * jh:8 * (jh + 1), 0:64],
                            ps.rearrange("q (t e) -> q t e", e=64))
                    ops.append(cp)

                def fin(nat=nat, p=p):
                    # bounce through DRAM: the write is a plain contiguous
                    # dump; the read's (rr t) partition dim has uniform
                    # stride 65 elems, so vh[16rr+t, 65bb+e] = nat[8bb+rr,t,e]
                    # in one 3-dim HWDGE DMA each way. On the sync queue these
                    # sit AFTER the weight-chunk loads, so they don't steal
                    # DMA bandwidth from the projection-critical stream.
                    nc.sync.dma_start(out=vsh[p], in_=nat)
                    nc.sync.dma_start(
                        out=vh[p],
                        in_=vsh[p].rearrange(
                            "(bb rr) (t e) -> (rr t) bb e", bb=16, e=65))
                ops.append(fin)
                return ops

            for i01 in range(2):
                for op in emit_vproj(i01, 0, pps, "ps2", 4):
                    op()

            # ------------- Q/K transposed projections (per group) ---------
            def emit_qkproj_cblock(wt, dstv, g, c, psum_pool, tag, psbufs):
                """Returns thunks: matmul chains + psum->qht strided copies."""
                ps = psum_pool.tile([128, 512], F32, tag=tag, bufs=psbufs,
                                    name=f"qkps{g}{c}")
                ops = []

                def chain(ps=ps):
                    # single accumulation chain; the moving operand carries
                    # both pairs as two free dims (i stride 1024, j stride 1)
                    for kb in range(8):
                        nc.tensor.matmul(
                            ps[:, 0:256],
                            lhsT=wt[kb][:, 128 * c:128 * (c + 1)],
                            rhs=xthg[g][:, :, kb, :],
                            start=(kb == 0), stop=(kb == 7),
                        )
                ops.append(chain)

                def copies(ps=ps):
                    psv = ps[:, 0:256].rearrange("q (i j) -> q i j", i=2)
                    for t2 in range(2):
                        for i in range(2):
                            dst = dstv[64 * i:64 * (i + 1), g, :, 2 * c + t2]
                            src = psv[64 * t2:64 * t2 + 64, i, :]
                            # g0 runs in the projection phase where Act is
                            # idle; g1 copies drain inside attention where
                            # Act is exp-bound, so they stay on DVE.
                            if g == 0 and not (t2 ^ i):
                                nc.scalar.copy(dst, src)
                            else:
                                nc.vector.tensor_copy(dst, src)
                ops.append(copies)
                return ops

            for wt, dstv in ((wqt, qv), (wkt, kv)):
                for c in range(8):
                    for op in emit_qkproj_cblock(wt, dstv, 0, c, pps,
                                                 "ps2", 4):
                        op()

            pps_ctx.__exit__(None, None, None)

            nc.sync.dma_start(out=wo_sb,
                              in_=wo.rearrange("(c k) j -> k c j", k=128))
            nc.sync.dma_start(out=bias_sb, in_=bias[:])

            # ------------- attention + output projection ------------------
            pending = []

            def drain(n):
                for _ in range(min(n, len(pending))):
                    pending.pop(0)()

            with tc.tile_pool(name="aps", bufs=1, space="PSUM") as aps:

                pvs_by_a = {}

                def emit_attention(g):
                    # software pipeline: PV for iteration k is emitted during
                    # iteration k+1, so the PE streams S matmuls without
                    # blocking on exp, and Act stays saturated.
                    iters = [(a, gg, i)
                             for a in range(4)
                             for gg in range(2 * a + 2)
                             for i in range(2)]

                    def emit_s(a, gg, i):
                        # Diagonal blocks (gg >= 2a): q2-half r = 2(gg-2a)+q2
                        # only has visible columns j >= 128r of the chunk.
                        # S computes cols >= min(128r, 256) (fp32r needs
                        # ap>=256), exp covers cols >= 128r, and the causal
                        # boundary is a single 128-wide triangular band
                        # handled by a small affine_select.
                        diag = gg >= 2 * a
                        st = aps.tile([128, 1024], F32, tag="sts",
                                      bufs=2, name="sts")
                        for q2 in range(2):
                            bb = 2 * gg + q2
                            r = 2 * (gg - 2 * a) + q2
                            c0 = min(128 * r, 256) if diag else 0
                            nc.tensor.matmul(
                                st[:, q2 * 512 + c0:(q2 + 1) * 512],
                                lhsT=kht[64 * i:64 * (i + 1), g,
                                         128 * bb:128 * (bb + 1)],
                                rhs=qht[64 * i:64 * (i + 1), g,
                                        512 * a + c0:512 * (a + 1)],
                                start=True, stop=True,
                            )
                        pt = ptp.tile([128, 1024], BF16, tag="pt", name="pt")
                        if not diag:
                            nc.scalar.activation(pt, st, EXP)
                        else:
                            for q2 in range(2):
                                r = 2 * (gg - 2 * a) + q2
                                e0 = q2 * 512 + 128 * r
                                nc.scalar.activation(
                                    pt[:, e0:(q2 + 1) * 512],
                                    st[:, e0:(q2 + 1) * 512], EXP)
                                # triangular boundary band: keep iff u >= q
                                nc.gpsimd.affine_select(
                                    out=pt[:, e0:e0 + 128],
                                    in_=pt[:, e0:e0 + 128],
                                    compare_op=mybir.AluOpType.is_ge,
                                    fill=0.0,
                                    base=0,
                                    pattern=[[1, 128]],
                                    channel_multiplier=-1,
                                )
                        return pt

                    def emit_pv(a, gg, i, pt):
                        diag = gg >= 2 * a
                        if gg == 0:
                            pvs_by_a[(g, a, i)] = aps.tile(
                                [65, 512], F32, tag=f"pv{i}", name=f"pv{i}")
                        pv = pvs_by_a[(g, a, i)]
                        for q2 in range(2):
                            bb = 2 * gg + q2
                            r = 2 * (gg - 2 * a) + q2
                            c0 = 128 * r if diag else 0
                            nc.tensor.matmul(
                                pv[:, c0:512],
                                lhsT=vh[2 * g + i][:, bb, :],
                                rhs=pt[:, q2 * 512 + c0:(q2 + 1) * 512],
                                start=(bb == 0), stop=(bb == 4 * a + 3),
                            )
                        if gg == 2 * a + 1:
                            # free the psum accumulator fast; finish
                            # normalization off the critical path
                            p = 2 * g + i
                            osb = rp.tile([65, 512], F32, tag="osb",
                                          bufs=2, name=f"osb{p}_{a}")
                            nc.vector.tensor_copy(osb, pv)
                            r1 = rp.tile([1, 512], F32, tag="r1", name="r1")
                            nc.vector.reciprocal(r1, osb[64:65, :])
                            rb = rp.tile([64, 512], F32, tag="rb", name="rb")
                            nc.gpsimd.partition_broadcast(rb, r1)
                            osv = osb.rearrange(
                                "q (r c s2) -> q c s2 r", c=8, s2=2)
                            rbv = rb.rearrange(
                                "q (r c s2) -> q c s2 r", c=8, s2=2)
                            onv = onorm[p].rearrange(
                                "q (c s2 r) -> q c s2 r", c=8, s2=2)
                            nc.vector.tensor_mul(
                                onv[0:64, :, 0, 32 * a:32 * (a + 1)],
                                osv[0:64, :, 0, :], rbv[:, :, 0, :])
                            nc.vector.tensor_mul(
                                onv[64:128, :, 0, 32 * a:32 * (a + 1)],
                                osv[0:64, :, 1, :], rbv[:, :, 1, :])

                    prev = None
                    for it in iters:
                        pt = emit_s(*it)
                        if prev is not None:
                            emit_pv(*prev[0], prev[1])
                        prev = (it, pt)
                        drain(2)
                    emit_pv(*prev[0], prev[1])

                def emit_y(g, inline=False):
                    for i in range(2):
                        p = 2 * g + i
                        onp_ = onorm[p]
                        ysb = ypool.tile([128, 1024], F32, tag="ysb",
                                         name=f"ysb{p}")
                        for jo in range(2):
                            yps = aps.tile([128, 512], F32, tag="aux",
                                           bufs=2, name=f"yps{p}_{jo}")

                            def mm(jo=jo, yps=yps, onp_=onp_):
                                for c in range(8):
                                    nc.tensor.matmul(
                                        yps,
                                        lhsT=onp_[:, 256 * c:256 * c + 128],
                                        rhs=wo_sb[:, c,
                                                  512 * jo:512 * (jo + 1)],
                                        start=(c == 0), stop=(c == 7),
                                    )

                            def fin(jo=jo, yps=yps, ysb=ysb, p=p):
                                nc.vector.tensor_add(
                                    ysb[:, 512 * jo:512 * (jo + 1)], yps,
                                    bias_sb[:, 512 * jo:512 * (jo + 1)])
                                if jo == 1:
                                    nc.sync.dma_start(out=out[p], in_=ysb)

                            if inline:
                                mm(); fin()
                            else:
                                pending.append(mm)
                                pending.append(fin)

                # group-1 Q/K projections + V pairs 2,3 drain inside
                # group-0 attention (aux psum, 1 bank each)
                for wt, dstv in ((wqt, qv), (wkt, kv)):
                    for c in range(8):
                        pending.extend(emit_qkproj_cblock(
                            wt, dstv, 1, c, aps, "aux", 2))
                for i01 in range(2):
                    pending.extend(emit_vproj(i01, 1, aps, "aux", 2))

                emit_attention(0)
                emit_y(0)
                emit_attention(1)
                emit_y(1, inline=True)
                drain(len(pending))

    nc.finalize()
    return nc


def _host_prep(input_seq_embs, W_Q, W_K, W_V, W_O, b_O):
    X = np.asarray(input_seq_embs, dtype=np.float32)
    wq_arr = np.ascontiguousarray(np.asarray(W_Q, dtype=np.float32).T).astype(np.float16)
    wk_arr = np.ascontiguousarray(np.asarray(W_K, dtype=np.float32).T).astype(np.float16)
    wv_arr = np.ascontiguousarray(np.asarray(W_V, dtype=np.float32).T).astype(np.float16)
    wo_arr = np.ascontiguousarray(
        np.asarray(W_O, dtype=np.float32).T).astype(ml_dtypes.bfloat16)
    bias_arr = np.ascontiguousarray(
        np.broadcast_to(np.asarray(b_O, dtype=np.float32), (128, 1024)))

    in_maps = []
    for c in range(NCORES):
        xts = []
        for p in range(PPC):
            g = PPC * c + p
            bb, hh = g // H, g % H
            # xth[p][k, 8*kb + j-block]: per-partition contiguous chunks of
            # X_s^T: xth[p][k, 128*kb + j] = X_s^T[128*kb + k, j]
            xs_t = X[bb, 128 * hh:128 * (hh + 1), :].T  # (1024, 128)
            xts.append(np.ascontiguousarray(
                xs_t.reshape(8, 128, 128).transpose(1, 0, 2).reshape(128, 1024)
            ).astype(np.float16))
        in_maps.append({
            "xth": np.stack(xts),
            "wq": wq_arr, "wk": wk_arr, "wv": wv_arr, "wo": wo_arr,
            "bias": bias_arr,
        })
    return in_maps


_CACHED_NC = None


def get_nc():
    global _CACHED_NC
    if _CACHED_NC is None:
        _CACHED_NC = build_nc()
    return _CACHED_NC


def kernel(**inputs) -> np.ndarray:
    nc = get_nc()
    in_maps = _host_prep(**inputs)
    res = run_bass_kernel_spmd(nc, in_maps, list(range(NCORES)))
    out = np.empty((B, L, D), dtype=np.float32)
    for c in range(NCORES):
        y = res.results[c]["out"]  # (4, 128, 1024)
        for p in range(PPC):
            g = PPC * c + p
            bb, hh = g // H, g % H
            out[bb, 128 * hh:128 * (hh + 1), :] = y[p]
    return out


# revision 37
# speedup vs baseline: 6.4465x; 1.1375x over previous
"""Trainium2 Bass kernel for nn_MultiHeadAttention_66322884984909.

Math (faithful to reference):
  Q = X @ W_Q.T reshaped (B, H, L, hd) via DIRECT reshape -> head h owns rows
  128h:128(h+1) of the projected (L, D) matrix, reinterpreted as (2048, 64).
  The 32 (batch, head) pairs are independent: 8 cores x 4 pairs, no
  collectives.

Per pair (X_s = X[b, 128h:128h+128, :], (128, 1024)):
  Qh = (X_s @ W_Q.T).reshape(2048, 64);  Kh, Vh likewise
  S  = Qh @ Kh.T  (2048x2048, causal, NO 1/sqrt(hd) scale, no max-sub)
  O  = softmax(S) @ Vh  -> Y = O.reshape(128, 1024) @ W_O.T + b_O

Implementation notes:
  * Q^T/K^T come straight out of transposed projections (lhsT = W-chunk,
    rhs = X^T, both fp16) written by strided psum->SBUF copies into the
    natural l-order layout qht[d, l] = Qh[l, d]. No PE transposes.
  * Projections are fp16 (X, W_Q/K/V); V/P/O/W_O are bf16; S stays fp32r.
    Measured: 3.2e-3 norm-rel error on HW (tolerance 2e-2).
  * V natural projection -> bf16 nat tile (with embedded ones columns for
    the row-sum trick) -> DRAM bounce: contiguous dump, then a single
    3-dim HWDGE read whose (rr t) partition dim has uniform stride, which
    lands vh[kpos, 65*bb + e] = nat[8*bb + rr, t, e].
  * Attention is software-pipelined: PV for iteration k is emitted during
    iteration k+1 so the PE streams S matmuls while Act (exp) runs; exp
    is the attention-phase bottleneck, so group-1 Q/K/V projections and
    group-0 output projections are deferred thunks drained into the
    Act-bound bubbles of the preceding group's attention.
  * Diagonal S blocks are restricted to their visible columns: S and PV
    skip fully-masked column ranges, exp covers only visible columns, and
    the causal boundary is a 128-wide triangular band zeroed by a small
    affine_select (per-half), keeping Pool off the critical path.
  * Normalization is deferred: PV psum is copied to SBUF so the psum
    accumulator frees immediately; recip/broadcast/mul run off the
    critical path. onorm is t-major with O^T and O^T-shifted-by-one in
    the two partition halves, so the output projection contracts 128
    deep in 8 steps with contiguous stationary APs and natural bf16
    W_O.T c-blocks.
  * Weights/X are loaded in per-128-row chunks into separate tiles so
    matmuls start as soon as their chunk lands; V bounces ride the sync
    queue behind the weight stream.
  * PSUM: 2x[128,1024] S tiles + 2 PV accumulators + 2x[128,512] aux
    (projection drains / output projection halves) = exactly 8 banks,
    one accumulation chain per bank.
"""

import numpy as np
import ml_dtypes

import concourse.bass as bass
from concourse import bacc
import concourse.mybir as mybir
import concourse.tile as tile
from concourse.bass_utils import run_bass_kernel_spmd

F32 = mybir.dt.float32
F32R = mybir.dt.float32r
BF16 = mybir.dt.bfloat16
FP16 = mybir.dt.float16
EXP = mybir.ActivationFunctionType.Exp

B, L, D = 2, 2048, 1024
H, HD = 16, 64
NCORES = 8
PPC = 4  # pairs per core
NG = PPC // 2  # pair groups


def build_nc(repeat=1):
    nc = bacc.Bacc(trn_type="TRN2", target_bir_lowering=False, debug=False)

    xth = nc.declare_dram_parameter("xth", [PPC, 128, 1024], FP16, isOutput=False)
    wq = nc.declare_dram_parameter("wq", [1024, 1024], FP16, isOutput=False)
    wk = nc.declare_dram_parameter("wk", [1024, 1024], FP16, isOutput=False)
    wv = nc.declare_dram_parameter("wv", [1024, 1024], FP16, isOutput=False)
    wo = nc.declare_dram_parameter("wo", [1024, 1024], BF16, isOutput=False)
    bias = nc.declare_dram_parameter("bias", [128, 1024], F32, isOutput=False)
    out = nc.declare_dram_parameter("out", [PPC, 128, 1024], F32, isOutput=True)
    vsh = nc.dram_tensor("vsh", [PPC, 128, 1040], BF16)

    with tile.TileContext(nc) as tc:
      for _rep in range(repeat):
        with (
            tc.tile_pool(name="consts", bufs=1) as consts,
            tc.tile_pool(name="wpool", bufs=1) as wpool,
            tc.tile_pool(name="qkp", bufs=1) as qkp,
            tc.tile_pool(name="vp", bufs=1) as vpool,
            tc.tile_pool(name="natp", bufs=2) as natp,
            tc.tile_pool(name="onp", bufs=1) as onp,
            tc.tile_pool(name="ptp", bufs=8) as ptp,
            tc.tile_pool(name="rp", bufs=2) as rp,
            tc.tile_pool(name="yp", bufs=2) as ypool,
        ):
            # ---- loads: X chunks per group, weights per-128-row chunk ----
            xthg = [consts.tile([128, 2, 8, 128], FP16, tag=f"xth{g}",
                                name=f"xthg{g}") for g in range(NG)]
            for g in range(NG):
                for i in range(2):
                    nc.scalar.dma_start(
                        out=xthg[g][:, i, :, :],
                        in_=xth[2 * g + i].rearrange("k (kb j) -> k kb j", j=128))

            wvt = [wpool.tile([128, 1024], FP16, tag=f"wv{kc}",
                              name=f"wvt{kc}") for kc in range(8)]
            wqt = [wpool.tile([128, 1024], FP16, tag=f"wq{kc}",
                              name=f"wqt{kc}") for kc in range(8)]
            wkt = [wpool.tile([128, 1024], FP16, tag=f"wk{kc}",
                              name=f"wkt{kc}") for kc in range(8)]
            for kc in range(8):
                nc.sync.dma_start(out=wvt[kc], in_=wv[128 * kc:128 * (kc + 1), :])
            for kc in range(8):
                nc.sync.dma_start(out=wqt[kc], in_=wq[128 * kc:128 * (kc + 1), :])
            for kc in range(8):
                nc.sync.dma_start(out=wkt[kc], in_=wk[128 * kc:128 * (kc + 1), :])

            bias_sb = consts.tile([128, 1024], F32)
            wo_sb = consts.tile([128, 8, 1024], BF16)

            qht = qkp.tile([128, NG, 2048], F32R, tag="qht", name="qht")
            kht = qkp.tile([128, NG, 2048], F32R, tag="kht", name="kht")
            vh = [vpool.tile([128, 16, 65], BF16, tag=f"vh{p}", name=f"vh{p}")
                  for p in range(PPC)]
            onorm = [onp.tile([128, 2048], BF16, tag=f"on{p}", name=f"on{p}")
                     for p in range(PPC)]
            # l-order columns viewed as (j, t): col l = 16j + t
            qv = qht.rearrange("p g (j t) -> p g j t", t=16)
            kv = kht.rearrange("p g (j t) -> p g j t", t=16)

            # preload the exp activation table during the projection phase
            scratch1 = consts.tile([1, 1], BF16, name="scr1")
            nc.scalar.activation(scratch1, xthg[0][0:1, 0, 0, 0:1], EXP)

            pps_ctx = tc.tile_pool(name="pps", bufs=4, space="PSUM")
            pps = pps_ctx.__enter__()

            # ------------- V projection + scatter -------------------------
            def emit_vproj(i01, g, psum_pool, tag, psbufs, psw=512):
                p = 2 * g + i01
                nat = natp.tile([128, 16, 65], BF16, tag="nat", bufs=2,
                                name=f"nat{p}")
                ops = []
                ops.append(lambda: nc.gpsimd.memset(nat[:, :, 64:65], 1.0))
                for jh in range(2):
                    ps = psum_pool.tile([128, psw], F32, tag=tag, bufs=psbufs,
                                        name=f"vps{p}_{jh}")

                    def mm8(jh=jh, ps=ps):
                        for kc in range(8):
                            nc.tensor.matmul(
                                ps[:, 0:512],
                                lhsT=xthg[g][:, i01, kc, :],
                                rhs=wvt[kc][:, jh * 512:(jh + 1) * 512],
                                start=(kc == 0), stop=(kc == 7),
                            )
                    ops.append(mm8)

                    def cp(jh=jh, ps=ps):
                        nc.vector.tensor_copy(
                            nat[:, 8 * jh:8 * (jh + 1), 0:64],
                            ps[:, 0:512].rearrange("q (t e) -> q t e", e=64))
                    ops.append(cp)

                def fin(nat=nat, p=p):
                    # bounce through DRAM: the write is a plain contiguous
                    # dump; the read's (rr t) partition dim has uniform
                    # stride 65 elems, so vh[16rr+t, 65bb+e] = nat[8bb+rr,t,e]
                    # in one 3-dim HWDGE DMA each way. On the sync queue these
                    # sit AFTER the weight-chunk loads, so they don't steal
                    # DMA bandwidth from the projection-critical stream.
                    nc.sync.dma_start(out=vsh[p], in_=nat)
                    nc.sync.dma_start(
                        out=vh[p],
                        in_=vsh[p].rearrange(
                            "(bb rr) (t e) -> (rr t) bb e", bb=16, e=65))
                ops.append(fin)
                return ops

            for i01 in range(2):
                for op in emit_vproj(i01, 0, pps, "ps2", 4):
                    op()

            # ------------- Q/K transposed projections (per group) ---------
            def emit_qkproj_cblock(wt, dstv, g, c, psum_pool, tag, psbufs,
                                   psw=512):
                """Returns thunks: matmul chains + psum->qht strided copies."""
                ps = psum_pool.tile([128, psw], F32, tag=tag, bufs=psbufs,
                                    name=f"qkps{g}{c}")
                ops = []

                def chain(ps=ps):
                    # single accumulation chain; the moving operand carries
                    # both pairs as two free dims (i stride 1024, j stride 1)
                    for kb in range(8):
                        nc.tensor.matmul(
                            ps[:, 0:256],
                            lhsT=wt[kb][:, 128 * c:128 * (c + 1)],
                            rhs=xthg[g][:, :, kb, :],
                            start=(kb == 0), stop=(kb == 7),
                        )
                ops.append(chain)

                def copies(ps=ps):
                    psv = ps[:, 0:256].rearrange("q (i j) -> q i j", i=2)
                    for t2 in range(2):
                        for i in range(2):
                            dst = dstv[64 * i:64 * (i + 1), g, :, 2 * c + t2]
                            src = psv[64 * t2:64 * t2 + 64, i, :]
                            # g0 runs in the projection phase where Act is
                            # idle; g1 copies drain inside attention where
                            # Act is exp-bound, so they stay on DVE.
                            if g == 0 and not (t2 ^ i):
                                nc.scalar.copy(dst, src)
                            else:
                                nc.vector.tensor_copy(dst, src)
                ops.append(copies)
                return ops

            for wt, dstv in ((wqt, qv), (wkt, kv)):
                for c in range(8):
                    for op in emit_qkproj_cblock(wt, dstv, 0, c, pps,
                                                 "ps2", 4):
                        op()

            pps_ctx.__exit__(None, None, None)

            nc.sync.dma_start(out=wo_sb,
                              in_=wo.rearrange("(c k) j -> k c j", k=128))
            nc.sync.dma_start(out=bias_sb, in_=bias[:])

            # ------------- attention + output projection ------------------
            pending = []

            def drain(n):
                for _ in range(min(n, len(pending))):
                    pending.pop(0)()

            with tc.tile_pool(name="aps", bufs=1, space="PSUM") as aps:

                pvs_by_a = {}

                def emit_attention(g):
                    # software pipeline: PV for iteration k is emitted during
                    # iteration k+1, so the PE streams S matmuls without
                    # blocking on exp, and Act stays saturated.
                    iters = [(a, gg, i)
                             for a in range(4)
                             for gg in range(2 * a + 2)
                             for i in range(2)]

                    def emit_s(a, gg, i):
                        # Diagonal blocks (gg >= 2a): q2-half r = 2(gg-2a)+q2
                        # only has visible columns j >= 128r of the chunk.
                        # S computes cols >= min(128r, 256) (fp32r needs
                        # ap>=256), exp covers cols >= 128r, and the causal
                        # boundary is a single 128-wide triangular band
                        # handled by a small affine_select.
                        diag = gg >= 2 * a
                        st = aps.tile([128, 1024], F32, tag="sts",
                                      bufs=2, name="sts")
                        for q2 in range(2):
                            bb = 2 * gg + q2
                            r = 2 * (gg - 2 * a) + q2
                            c0 = min(128 * r, 256) if diag else 0
                            nc.tensor.matmul(
                                st[:, q2 * 512 + c0:(q2 + 1) * 512],
                                lhsT=kht[64 * i:64 * (i + 1), g,
                                         128 * bb:128 * (bb + 1)],
                                rhs=qht[64 * i:64 * (i + 1), g,
                                        512 * a + c0:512 * (a + 1)],
                                start=True, stop=True,
                            )
                        pt = ptp.tile([128, 1024], BF16, tag="pt", name="pt")
                        if not diag:
                            nc.scalar.activation(pt, st, EXP)
                        else:
                            for q2 in range(2):
                                r = 2 * (gg - 2 * a) + q2
                                e0 = q2 * 512 + 128 * r
                                nc.scalar.activation(
                                    pt[:, e0:(q2 + 1) * 512],
                                    st[:, e0:(q2 + 1) * 512], EXP)
                                # triangular boundary band: keep iff u >= q
                                nc.gpsimd.affine_select(
                                    out=pt[:, e0:e0 + 128],
                                    in_=pt[:, e0:e0 + 128],
                                    compare_op=mybir.AluOpType.is_ge,
                                    fill=0.0,
                                    base=0,
                                    pattern=[[1, 128]],
                                    channel_multiplier=-1,
                                )
                        return pt

                    def emit_pv(a, gg, i, pt):
                        diag = gg >= 2 * a
                        if gg == 0:
                            pvs_by_a[(g, a, i)] = aps.tile(
                                [65, 512], F32, tag=f"pv{i}", name=f"pv{i}")
                        pv = pvs_by_a[(g, a, i)]
                        for q2 in range(2):
                            bb = 2 * gg + q2
                            r = 2 * (gg - 2 * a) + q2
                            c0 = 128 * r if diag else 0
                            nc.tensor.matmul(
                                pv[:, c0:512],
                                lhsT=vh[2 * g + i][:, bb, :],
                                rhs=pt[:, q2 * 512 + c0:(q2 + 1) * 512],
                                start=(bb == 0), stop=(bb == 4 * a + 3),
                            )
                        if gg == 2 * a + 1:
                            # free the psum accumulator fast; finish
                            # normalization off the critical path
                            p = 2 * g + i
                            osb = rp.tile([65, 512], F32, tag="osb",
                                          bufs=3, name=f"osb{p}_{a}")
                            nc.vector.tensor_copy(osb, pv)
                            r1 = rp.tile([1, 512], F32, tag="r1", name="r1")
                            nc.vector.reciprocal(r1, osb[64:65, :])
                            rb = rp.tile([64, 512], F32, tag="rb", name="rb")
                            nc.gpsimd.partition_broadcast(rb, r1)
                            osv = osb.rearrange(
                                "q (r c s2) -> q c s2 r", c=8, s2=2)
                            rbv = rb.rearrange(
                                "q (r c s2) -> q c s2 r", c=8, s2=2)
                            onv = onorm[p].rearrange(
                                "q (c s2 r) -> q c s2 r", c=8, s2=2)
                            nc.vector.tensor_mul(
                                onv[0:64, :, 0, 32 * a:32 * (a + 1)],
                                osv[0:64, :, 0, :], rbv[:, :, 0, :])
                            nc.vector.tensor_mul(
                                onv[64:128, :, 0, 32 * a:32 * (a + 1)],
                                osv[0:64, :, 1, :], rbv[:, :, 1, :])

                    prev = None
                    for it in iters:
                        pt = emit_s(*it)
                        if prev is not None:
                            emit_pv(*prev[0], prev[1])
                        prev = (it, pt)
                        drain(2)
                    emit_pv(*prev[0], prev[1])

                def emit_y(g, inline=False):
                    for i in range(2):
                        p = 2 * g + i
                        onp_ = onorm[p]
                        ysb = ypool.tile([128, 1024], F32, tag="ysb",
                                         name=f"ysb{p}")
                        yps = None

                        for jo in range(2):
                            yps = aps.tile([128, 512], F32, tag="aux",
                                           bufs=2, name=f"yps{p}_{jo}")

                            def mm(jo=jo, yps=yps, onp_=onp_):
                                for c in range(8):
                                    nc.tensor.matmul(
                                        yps,
                                        lhsT=onp_[:, 256 * c:256 * c + 128],
                                        rhs=wo_sb[:, c,
                                                  512 * jo:512 * (jo + 1)],
                                        start=(c == 0), stop=(c == 7),
                                    )

                            def fin(jo=jo, yps=yps, ysb=ysb, p=p):
                                nc.vector.tensor_add(
                                    ysb[:, 512 * jo:512 * (jo + 1)], yps,
                                    bias_sb[:, 512 * jo:512 * (jo + 1)])
                                if jo == 1:
                                    nc.sync.dma_start(out=out[p], in_=ysb)

                            if inline:
                                mm(); fin()
                            else:
                                pending.append(mm)
                                pending.append(fin)

                # group-1 Q/K projections + V pairs 2,3 drain inside
                # group-0 attention (aux psum, 1 bank each)
                for wt, dstv in ((wqt, qv), (wkt, kv)):
                    for c in range(8):
                        pending.extend(emit_qkproj_cblock(
                            wt, dstv, 1, c, aps, "aux", 2))
                for i01 in range(2):
                    pending.extend(emit_vproj(i01, 1, aps, "aux", 2))

                emit_attention(0)
                emit_y(0)
                emit_attention(1)
                emit_y(1, inline=True)
                drain(len(pending))

    nc.finalize()
    return nc


def _host_prep(input_seq_embs, W_Q, W_K, W_V, W_O, b_O):
    X = np.asarray(input_seq_embs, dtype=np.float32)
    wq_arr = np.ascontiguousarray(np.asarray(W_Q, dtype=np.float32).T).astype(np.float16)
    wk_arr = np.ascontiguousarray(np.asarray(W_K, dtype=np.float32).T).astype(np.float16)
    wv_arr = np.ascontiguousarray(np.asarray(W_V, dtype=np.float32).T).astype(np.float16)
    wo_arr = np.ascontiguousarray(
        np.asarray(W_O, dtype=np.float32).T).astype(ml_dtypes.bfloat16)
    bias_arr = np.ascontiguousarray(
        np.broadcast_to(np.asarray(b_O, dtype=np.float32), (128, 1024)))

    in_maps = []
    for c in range(NCORES):
        xts = []
        for p in range(PPC):
            g = PPC * c + p
            bb, hh = g // H, g % H
            # xth[p][k, 8*kb + j-block]: per-partition contiguous chunks of
            # X_s^T: xth[p][k, 128*kb + j] = X_s^T[128*kb + k, j]
            xs_t = X[bb, 128 * hh:128 * (hh + 1), :].T  # (1024, 128)
            xts.append(np.ascontiguousarray(
                xs_t.reshape(8, 128, 128).transpose(1, 0, 2).reshape(128, 1024)
            ).astype(np.float16))
        in_maps.append({
            "xth": np.stack(xts),
            "wq": wq_arr, "wk": wk_arr, "wv": wv_arr, "wo": wo_arr,
            "bias": bias_arr,
        })
    return in_maps


_CACHED_NC = None


def get_nc():
    global _CACHED_NC
    if _CACHED_NC is None:
        _CACHED_NC = build_nc()
    return _CACHED_NC


def kernel(**inputs) -> np.ndarray:
    nc = get_nc()
    in_maps = _host_prep(**inputs)
    res = run_bass_kernel_spmd(nc, in_maps, list(range(NCORES)))
    out = np.empty((B, L, D), dtype=np.float32)
    for c in range(NCORES):
        y = res.results[c]["out"]  # (4, 128, 1024)
        for p in range(PPC):
            g = PPC * c + p
            bb, hh = g // H, g % H
            out[bb, 128 * hh:128 * (hh + 1), :] = y[p]
    return out
